# revision 26
# baseline (speedup 1.0000x reference)
"""BCH/RS systematic encoder kernel for Trainium2 (8 NeuronCores, data parallel).

Computes out = concat([msg, (msg @ Gp) mod 2], axis=-1) for
msg [16384, 1000] f32 of 0/1 bits and Gp [1000, 256] f32 of 0/1 bits.

Design (per core, 2048 rows, 16 chunks of 128):
  - SWDGE cast-load msg chunk f32 -> bf16 SBUF (0/1 exact in bf16)
  - SWDGE cast-store bf16 -> f32 to out[:, :1000] (systematic copy-through)
  - DMA xbar transpose (2-byte) 128x128 blocks: msg natural -> msgT [k, m]
  - 8 accumulating bf16 matmuls: psum[m,256] += msgT_k.T @ Gp_k (fp32 accum, exact)
  - DVE tensor_scalar mod 2.0 on psum -> SBUF f32
  - store parity to out[:, 1000:1256]
HBM traffic/core = 8.19 MB read + 10.29 MB write (the minimum).
"""

import os
import sys

import numpy as np

if os.path.isdir("/opt/trn_rl_repo") and "/opt/trn_rl_repo" not in sys.path:
    sys.path.insert(0, "/opt/trn_rl_repo")

import ml_dtypes

import concourse.bacc as bacc
import concourse.mybir as mybir
import concourse.tile as tile
from concourse.bass_utils import run_bass_kernel_spmd

BATCH = 16384
MSG = 1000
NPAR = 256
NCORES = 8
ROWS = BATCH // NCORES  # 2048
P = 128
KCH = 8  # k chunks; padded K = 1024
KPAD = KCH * P

# test.py pokes these for profiling
TRACE = False
LAST_RESULT = None

_CACHE = {}


def build_nc(rows=ROWS):
    """Emit the Bass/Tile IR for one core handling `rows` rows."""
    mch = rows // P
    nc = bacc.Bacc("TRN2", target_bir_lowering=False, debug=False)
    msg = nc.dram_tensor("msg", [rows, MSG], mybir.dt.float32, kind="ExternalInput")
    gp = nc.dram_tensor("gp", [P, KCH * NPAR], mybir.dt.bfloat16, kind="ExternalInput")
    out = nc.dram_tensor(
        "out", [rows, MSG + NPAR], mybir.dt.float32, kind="ExternalOutput"
    )

    SC = 2  # m-chunks per superchunk (SWDGE/DVE batching granularity)
    n_super = mch // SC
    PRE = 2  # loads run this many superchunks ahead of compute
    LAG = 2  # parity stores trail compute by this many superchunks
    msg3 = msg[:, :].rearrange("(s c p) k -> s c p k", c=SC, p=P)
    out3 = out[:, :].rearrange("(s c p) k -> s c p k", c=SC, p=P)

    with tile.TileContext(nc) as tc:
        with (
            tc.tile_pool(name="gpool", bufs=1) as gpool,
            # a lives load(it)..store(it+PRE+LAG): span PRE+LAG+1 slots; extra
            # slots beyond that are what lets loads genuinely run ahead
            tc.tile_pool(name="apool", bufs=PRE + LAG + 5) as apool,
            tc.tile_pool(name="bpool", bufs=4) as bpool,
            tc.tile_pool(name="cpool", bufs=3) as cpool,
            tc.tile_pool(name="epool", bufs=LAG + 2) as epool,
            tc.tile_pool(name="ppool", bufs=6, space="PSUM") as ppool,
        ):
            # Gp resident in SBUF: gsb[q, kb*256 + n] = Gp_padded[kb*128 + q, n]
            gsb = gpool.tile([P, KCH * NPAR], mybir.dt.bfloat16)
            nc.sync.dma_start(out=gsb[:, :], in_=gp[:, :])

            a_tiles = {}
            es = {}

            # row stride must keep every a[:, c, :] slice 32B-aligned for the
            # xbar transpose: 1264 bf16 = 2528 B = 79*32
            ROWP = 1264

            def emit_load(si):
                # full output row in bf16: cols 0:1000 msg, 1000:1256 parity.
                # No zero-pad memset: the last k-chunk matmul contracts K=104,
                # so the PE never reads the transposed garbage rows.
                a = apool.tile([P, SC, ROWP], mybir.dt.bfloat16, tag="a")
                nc.gpsimd.dma_start(
                    out=a[:, :, 0:MSG], in_=msg3[si, :, :, :].rearrange("c p k -> p c k")
                )
                a_tiles[si] = a

            def emit_compute(si):
                a = a_tiles[si]
                # per-chunk xbar transpose: b[q, c*KCH + kb, p] = a[p, c, kb*128+q]
                # all on ONE HWDGE ring: concurrent xbar transposes from two
                # rings corrupt each other (shared xbar; this Tile does not
                # cross-engine-serialize them)
                b = bpool.tile([P, SC * KCH, P], mybir.dt.bfloat16, tag="b")
                for c in range(SC):
                    nc.sync.dma_start(
                        out=b[:, c * KCH : (c + 1) * KCH, :],
                        in_=a[:, c, 0:KPAD],
                        transpose=True,
                    )
                # both chunks accumulate side by side in one PSUM bank
                acc = ppool.tile([P, SC * NPAR], mybir.dt.float32, tag="acc")
                for c in range(SC):
                    for kb in range(KCH):
                        kk = P if kb < KCH - 1 else MSG - (KCH - 1) * P  # 104 tail
                        nc.tensor.matmul(
                            acc[:, c * NPAR : (c + 1) * NPAR],
                            b[0:kk, c * KCH + kb, :],
                            gsb[0:kk, kb * NPAR : (kb + 1) * NPAR],
                            start=(kb == 0),
                            stop=(kb == KCH - 1),
                        )
                # exact-integer f32 -> i32 eviction in ONE op on idle ACT
                c_i32 = cpool.tile([P, SC, NPAR], mybir.dt.int32, tag="c")
                nc.scalar.copy(
                    c_i32[:, :, :].rearrange("p c n -> p (c n)"), acc[:, :]
                )
                # mod 2 == AND 1 (bitVec op cannot cast, keep i32)
                e = epool.tile([P, SC, NPAR], mybir.dt.int32, tag="e")
                nc.vector.tensor_scalar(
                    e[:, :, :], c_i32[:, :, :], 1, None, mybir.AluOpType.bitwise_and
                )
                # parity into the output-row tile (0/1 exact in bf16)
                nc.vector.tensor_copy(a[:, :, MSG : MSG + NPAR], e[:, :, :])

            def emit_store(si):
                # single cast-store of the full rows: [p, c, 1256] bf16 -> f32
                a = a_tiles.pop(si)
                nc.gpsimd.dma_start(
                    out=out3[si, :, :, :].rearrange("c p k -> p c k"),
                    in_=a[:, :, 0 : MSG + NPAR],
                )

            for it in range(n_super + PRE + LAG):
                if it < n_super:
                    emit_load(it)
                j = it - PRE
                if 0 <= j < n_super:
                    emit_compute(j)
                k = it - PRE - LAG
                if 0 <= k < n_super:
                    emit_store(k)

    nc.compile()
    return nc


def prep_gp(Gp):
    """Pad Gp to 1024 rows and swizzle to the [128, 8*256] bf16 SBUF layout."""
    gp = np.asarray(Gp, dtype=np.float32)
    gp_pad = np.zeros((KPAD, NPAR), dtype=np.float32)
    gp_pad[:MSG] = gp
    gsw = gp_pad.reshape(KCH, P, NPAR).transpose(1, 0, 2).reshape(P, KCH * NPAR)
    return np.ascontiguousarray(gsw).astype(ml_dtypes.bfloat16)


def kernel(message_bits, Gp):
    global LAST_RESULT
    msg = np.ascontiguousarray(np.asarray(message_bits, dtype=np.float32))
    assert msg.shape == (BATCH, MSG), msg.shape
    gsw = prep_gp(Gp)

    if "nc" not in _CACHE:
        _CACHE["nc"] = build_nc()
    nc = _CACHE["nc"]

    in_maps = [
        {"msg": msg[i * ROWS : (i + 1) * ROWS], "gp": gsw} for i in range(NCORES)
    ]
    res = run_bass_kernel_spmd(
        nc, in_maps, core_ids=list(range(NCORES)), trace=TRACE
    )
    LAST_RESULT = res
    return np.concatenate([r["out"] for r in res.results], axis=0)
